# revision 1
# baseline (speedup 1.0000x reference)
"""GATv2 stack (3 layers + MLP head) on 8 Trainium2 NeuronCores.

Self-contained: takes full inputs, shards internally (dst-range node
partition), runs one SPMD Bass kernel on cores 0-7, returns full output.
"""
import sys

sys.path.insert(0, "/opt/trn_rl_repo")

import hashlib

import numpy as np
import ml_dtypes

import concourse.bass as bass
import concourse.tile as tile
from concourse import bacc, mybir
from concourse.bass_utils import run_bass_kernel_spmd

AF = mybir.ActivationFunctionType
ALU = mybir.AluOpType
F32 = mybir.dt.float32
BF16 = mybir.dt.bfloat16
I16 = mybir.dt.int16
BF_NP = ml_dtypes.bfloat16

P = 128
D = 128
DOUT = 64
N = 50000
NP_ = 50176            # padded nodes: 8 * 49 * 128
PC = 6272              # nodes per core
NST = 49               # super-tiles (128-dst blocks) per core
NCORE = 8
LO = 32768             # xl table split for int16 gather indices
NEG = 0.2
NLAYER = 3

import os as _os

# edge-stage dtype knob: F32 (safe) or BF16 (fast)
EDT = F32 if _os.environ.get("GAT_EDT", "bf16") == "f32" else BF16
EDT_NP = BF_NP if EDT is BF16 else np.float32
# matmul dtype for layers 1,2 node-level matmuls (exchange dtype is bf16)
XDT = BF16
XDT_NP = BF_NP

_CACHE = {}


def _wrap_idx(a):
    """[n] int -> [128, n//16] int16 wrapped (col-major over 16 parts, 8x tiled)."""
    a = a.astype(np.int16)
    arr16 = a.reshape(-1, 16).T
    return np.tile(arr16, (8, 1))


def _prep_edges(edge_index):
    src = np.asarray(edge_index[0], dtype=np.int64)
    dst = np.asarray(edge_index[1], dtype=np.int64)
    core = dst // PC
    stl = (dst % PC) // P
    key = core * NST + stl
    order = np.argsort(key, kind="stable")
    src_s, dst_s, key_s = src[order], dst[order], key[order]
    counts = np.bincount(key_s, minlength=NCORE * NST).reshape(NCORE, NST)
    starts = np.zeros(NCORE * NST + 1, np.int64)
    np.cumsum(counts.ravel(), out=starts[1:])

    T = np.ceil(counts.max(axis=0) / P).astype(np.int64)   # [NST]
    T = np.maximum(T, 1)
    CT = int(T.sum())

    srcidx = np.zeros((NCORE, CT * P), np.int64)
    xridx = np.zeros((NCORE, CT * P), np.int64)
    dstloc = np.full((NCORE, CT * P), -1.0, np.float32)
    off_t = np.concatenate([[0], np.cumsum(T)]) * P

    for c in range(NCORE):
        for s in range(NST):
            k = c * NST + s
            sl = slice(starts[k], starts[k + 1])
            n = starts[k + 1] - starts[k]
            base = off_t[s]
            srcidx[c, base:base + n] = src_s[sl]
            xridx[c, base:base + n] = dst_s[sl] - c * PC
            dstloc[c, base:base + n] = dst_s[sl] % P

    def pack(arr, dt):
        # edge slot i -> [i % P, off + i // P]
        return np.stack([arr[c].reshape(-1, P).T.copy().astype(dt)
                         for c in range(NCORE)])

    return {
        "T": T,
        "srcidx": pack(srcidx, np.int32),   # [NCORE, 128, CT] i32
        "xridx": pack(xridx, np.int32),
        "dstloc": pack(dstloc, np.float32),
    }


def _build_program(T):
    nc = bacc.Bacc("TRN2", target_bir_lowering=False, debug=False,
                   enable_asserts=True, num_devices=NCORE)
    CT = int(T.sum())

    dram = lambda n, s, d, **kw: nc.dram_tensor(n, s, d, **kw).ap()
    # ---- external inputs ----
    xT0 = dram("xT0", [P, NP_], F32, kind="ExternalInput")
    xT0own = dram("xT0own", [P, PC], F32, kind="ExternalInput")
    e_srcidx = dram("srcidx", [P, CT], mybir.dt.int32, kind="ExternalInput")
    e_xridx = dram("xridx", [P, CT], mybir.dt.int32, kind="ExternalInput")
    e_dstloc = dram("dstloc", [P, CT], EDT, kind="ExternalInput")
    wlt0 = dram("wlt0", [P, D], F32, kind="ExternalInput")
    wrt0 = dram("wrt0", [P, D], F32, kind="ExternalInput")
    wltb = dram("wltb", [2, P, D], XDT, kind="ExternalInput")
    wrtb = dram("wrtb", [2, P, D], XDT, kind="ExternalInput")
    blrow0 = dram("blrow0", [1, D], F32, kind="ExternalInput")
    brrow0 = dram("brrow0", [1, D], F32, kind="ExternalInput")
    blrowb = dram("blrowb", [2, 1, D], XDT, kind="ExternalInput")
    brrowb = dram("brrowb", [2, 1, D], XDT, kind="ExternalInput")
    att_bc = dram("att_bc", [NLAYER, P, D], EDT, kind="ExternalInput")
    biascol = dram("biascol", [NLAYER, P, 1], F32, kind="ExternalInput")
    w1t = dram("w1t", [P, D], F32, kind="ExternalInput")
    b1row = dram("b1row", [1, D], F32, kind="ExternalInput")
    w2t = dram("w2t", [P, DOUT], F32, kind="ExternalInput")
    b2row = dram("b2row", [1, DOUT], F32, kind="ExternalInput")
    iota_in = dram("iota_in", [P, P], EDT, kind="ExternalInput")
    ident_in = dram("ident_in", [P, P], F32, kind="ExternalInput")
    onescol_in = dram("onescol_in", [P, 1], EDT, kind="ExternalInput")
    onesrow0 = dram("onesrow0", [1, P], F32, kind="ExternalInput")
    onesrowb = dram("onesrowb", [1, P], XDT, kind="ExternalInput")
    onesrowe_in = dram("onesrowe", [1, P], EDT, kind="ExternalInput")
    epsone_in = dram("epsone", [1, 1], EDT, kind="ExternalInput")

    # ---- internal DRAM ----
    xl = [dram(f"xl{i}", [NP_, D], EDT) for i in range(NLAYER)]
    xr = [dram(f"xr{i}", [PC, D], EDT) for i in range(NLAYER)]
    xoTb = [dram(f"xoT{i}b", [P, PC], XDT) for i in range(2)]
    xTg = [dram(f"xTg{i}", [NCORE * P, PC], XDT, addr_space="Shared")
           for i in range(2)]
    xoT2 = dram("xoT2", [P, PC], F32)
    yT = dram("yT", [DOUT, PC], F32, kind="ExternalOutput")

    SLAB = 7 * P  # 896 nodes per xT slab DMA

    with tile.TileContext(nc) as tc:
        with (
            tc.tile_pool(name="const", bufs=1) as cpool,
            tc.tile_pool(name="wts", bufs=1) as wpool,
            tc.tile_pool(name="slab", bufs=3) as slabp,
            tc.tile_pool(name="nodeio", bufs=4) as niop,
            tc.tile_pool(name="idx", bufs=3) as idxp,
            tc.tile_pool(name="gath", bufs=2) as gathp,
            tc.tile_pool(name="edge", bufs=4) as edgep,
            tc.tile_pool(name="stt", bufs=3) as sttp,
            tc.tile_pool(name="epi", bufs=3) as epip,
            tc.tile_pool(name="psA", bufs=2, space="PSUM") as psA,
            tc.tile_pool(name="psE", bufs=2, space="PSUM") as psE,
            tc.tile_pool(name="psT", bufs=2, space="PSUM") as psT,
        ):
            # constants
            iota_t = cpool.tile([P, P], EDT)
            nc.sync.dma_start(out=iota_t[:], in_=iota_in[:])
            ident_t = cpool.tile([P, P], F32)
            nc.sync.dma_start(out=ident_t[:], in_=ident_in[:])
            onescol_t = cpool.tile([P, 1], EDT)
            nc.sync.dma_start(out=onescol_t[:], in_=onescol_in[:])
            onesrow0_t = cpool.tile([1, P], F32)
            nc.sync.dma_start(out=onesrow0_t[:], in_=onesrow0[:])
            onesrowb_t = cpool.tile([1, P], XDT)
            nc.sync.dma_start(out=onesrowb_t[:], in_=onesrowb[:])
            onesrowe_t = cpool.tile([1, P], EDT)
            nc.sync.dma_start(out=onesrowe_t[:], in_=onesrowe_in[:])
            epsone_t = cpool.tile([1, 1], EDT)
            nc.sync.dma_start(out=epsone_t[:], in_=epsone_in[:])

            off_t = np.concatenate([[0], np.cumsum(T)]).astype(int)

            def node_matmul_phase(src_ap, src_own_ap, dt_mm, wl_ap, wr_ap,
                                  bl_ap, br_ap, ones_t, xl_out, xr_out, li):
                """xl table (all nodes) and xr table (own nodes)."""
                wl_t = wpool.tile([P, D], dt_mm, tag=f"wl{li}")
                nc.sync.dma_start(out=wl_t[:], in_=wl_ap)
                wr_t = wpool.tile([P, D], dt_mm, tag=f"wr{li}")
                nc.sync.dma_start(out=wr_t[:], in_=wr_ap)
                bl_t = wpool.tile([1, D], dt_mm, tag=f"bl{li}")
                nc.sync.dma_start(out=bl_t[:], in_=bl_ap)
                br_t = wpool.tile([1, D], dt_mm, tag=f"br{li}")
                nc.sync.dma_start(out=br_t[:], in_=br_ap)

                # xl for all NP_ nodes
                for c in range(NCORE):
                    for sl in range(7):
                        st = slabp.tile([P, SLAB], dt_mm, tag="xslab")
                        col0 = sl * SLAB
                        if src_ap is xT0:
                            nc.sync.dma_start(
                                out=st[:], in_=xT0[:, c * PC + col0: c * PC + col0 + SLAB])
                        else:
                            nc.sync.dma_start(
                                out=st[:],
                                in_=src_ap[c * P:(c + 1) * P, col0:col0 + SLAB])
                        for t in range(7):
                            j = c * 49 + sl * 7 + t
                            ps = psA.tile([P, D], F32, tag="psA")
                            nc.tensor.matmul(out=ps[:], lhsT=st[:, t * P:(t + 1) * P],
                                             rhs=wl_t[:], start=True, stop=False)
                            nc.tensor.matmul(out=ps[:], lhsT=ones_t[:], rhs=bl_t[:],
                                             start=False, stop=True)
                            ot = niop.tile([P, D], EDT, tag="xlout")
                            nc.scalar.activation(ot[:], ps[:], AF.Copy)
                            nc.sync.dma_start(out=xl_out[j * P:(j + 1) * P, :], in_=ot[:])
                # xr for own PC nodes
                for sl in range(7):
                    st = slabp.tile([P, SLAB], dt_mm, tag="xslab")
                    nc.sync.dma_start(out=st[:], in_=src_own_ap[:, sl * SLAB:(sl + 1) * SLAB])
                    for t in range(7):
                        jj = sl * 7 + t
                        ps = psA.tile([P, D], F32, tag="psA")
                        nc.tensor.matmul(out=ps[:], lhsT=st[:, t * P:(t + 1) * P],
                                         rhs=wr_t[:], start=True, stop=False)
                        nc.tensor.matmul(out=ps[:], lhsT=ones_t[:], rhs=br_t[:],
                                         start=False, stop=True)
                        ot = niop.tile([P, D], EDT, tag="xlout")
                        nc.scalar.activation(ot[:], ps[:], AF.Copy)
                        nc.sync.dma_start(out=xr_out[jj * P:(jj + 1) * P, :], in_=ot[:])

            def edge_phase(li, xl_ap, xr_ap, out_own_ap, out_dt):
                att_t = wpool.tile([P, D], EDT, tag=f"att{li}")
                nc.sync.dma_start(out=att_t[:], in_=att_bc[li])
                bias_t = wpool.tile([P, 1], F32, tag=f"bias{li}")
                nc.sync.dma_start(out=bias_t[:], in_=biascol[li])

                nst = int(_os.environ.get("GAT_NST", str(NST)))
                for s in range(nst):
                    tt = int(T[s])
                    # index slices for this super-tile
                    is_t = idxp.tile([P, tt], mybir.dt.int32, tag="is")
                    nc.sync.dma_start(
                        out=is_t[:], in_=e_srcidx[:, off_t[s]:off_t[s] + tt])
                    ir_t = idxp.tile([P, tt], mybir.dt.int32, tag="ir")
                    nc.sync.dma_start(
                        out=ir_t[:], in_=e_xridx[:, off_t[s]:off_t[s] + tt])
                    dl_t = idxp.tile([P, tt], EDT, tag="dl")
                    nc.sync.dma_start(out=dl_t[:], in_=e_dstloc[:, off_t[s]:off_t[s] + tt])

                    xlbuf = gathp.tile([P, tt, D], EDT, tag="xlbuf")
                    xrbuf = gathp.tile([P, tt, D], EDT, tag="xrbuf")
                    for t in range(tt):
                        nc.gpsimd.indirect_dma_start(
                            out=xlbuf[:, t, :], out_offset=None, in_=xl_ap[:],
                            in_offset=bass.IndirectOffsetOnAxis(
                                ap=is_t[:, t:t + 1], axis=0))
                        nc.gpsimd.indirect_dma_start(
                            out=xrbuf[:, t, :], out_offset=None, in_=xr_ap[:],
                            in_offset=bass.IndirectOffsetOnAxis(
                                ap=ir_t[:, t:t + 1], axis=0))

                    logits_t = edgep.tile([P, tt], F32, tag="logits")
                    for t in range(tt):
                        xlg = xlbuf[:, t, :]
                        xrg = xrbuf[:, t, :]
                        t1 = sttp.tile([P, D], EDT, tag="t1")
                        nc.vector.tensor_add(t1[:], xlg, xrg)
                        lr = sttp.tile([P, D], EDT, tag="lr")
                        nc.vector.scalar_tensor_tensor(
                            out=lr[:], in0=t1[:], scalar=NEG, in1=t1[:],
                            op0=ALU.mult, op1=ALU.max)
                        junk = sttp.tile([P, D], EDT, tag="junk")
                        nc.vector.scalar_tensor_tensor(
                            out=junk[:], in0=lr[:], scalar=1.0, in1=att_t[:],
                            op0=ALU.mult, op1=ALU.mult,
                            accum_out=logits_t[:, t:t + 1])
                    ex_t = edgep.tile([P, tt], EDT, tag="ex")
                    nc.scalar.activation(ex_t[:], logits_t[:], AF.Exp)

                    psf = psE.tile([P, D], F32, tag="psf")
                    psd = psE.tile([P, 1], F32, tag="psd")
                    for t in range(tt):
                        selx = edgep.tile([P, P], EDT, tag="selx")
                        nc.vector.scalar_tensor_tensor(
                            out=selx[:], in0=iota_t[:], scalar=dl_t[:, t:t + 1],
                            in1=ex_t[:, t:t + 1].to_broadcast([P, P]),
                            op0=ALU.is_equal, op1=ALU.mult)
                        nc.tensor.matmul(out=psf[:], lhsT=selx[:],
                                         rhs=xlbuf[:, t, :],
                                         start=(t == 0), stop=(t == tt - 1))
                        nc.tensor.matmul(out=psd[:], lhsT=selx[:],
                                         rhs=onescol_t[:],
                                         start=(t == 0), stop=False)
                    nc.tensor.matmul(out=psd[:], lhsT=onesrowe_t[:],
                                     rhs=epsone_t[:], start=False, stop=True)
                    # epilogue
                    rec_t = epip.tile([P, 1], F32, tag="rec")
                    nc.vector.reciprocal(rec_t[:], psd[:])
                    outn = epip.tile([P, D], F32, tag="outn")
                    nc.scalar.activation(outn[:], psf[:], AF.Copy,
                                         scale=rec_t[:])
                    tps = psT.tile([P, D], F32, tag="psT")
                    nc.tensor.transpose(out=tps[:], in_=outn[:], identity=ident_t[:])
                    outT = epip.tile([P, D], out_dt, tag="outT")
                    nc.scalar.activation(outT[:], tps[:], AF.Relu, bias=bias_t[:])
                    nc.sync.dma_start(
                        out=out_own_ap[:, s * P:(s + 1) * P], in_=outT[:])

            # ---------------- layers ----------------
            import os as _os
            n_layers = int(_os.environ.get("GAT_LAYERS", str(NLAYER)))
            no_cc = bool(int(_os.environ.get("GAT_NO_CC", "0")))
            no_edge = bool(int(_os.environ.get("GAT_NO_EDGE", "0")))
            for li in range(n_layers):
                if li == 0:
                    node_matmul_phase(xT0, xT0own, F32, wlt0[:], wrt0[:],
                                      blrow0[:], brrow0[:], onesrow0_t,
                                      xl[0], xr[0], 0)
                else:
                    node_matmul_phase(xTg[li - 1], xoTb[li - 1], XDT,
                                      wltb[li - 1], wrtb[li - 1],
                                      blrowb[li - 1], brrowb[li - 1],
                                      onesrowb_t, xl[li], xr[li], li)
                if li < n_layers - 1 or n_layers < NLAYER:
                    if not no_edge:
                        edge_phase(li, xl[li], xr[li], xoTb[min(li, 1)], XDT)
                    if not no_cc:
                        nc.gpsimd.collective_compute(
                            "AllGather", ALU.bypass,
                            replica_groups=[list(range(NCORE))],
                            ins=[xoTb[min(li, 1)][:]], outs=[xTg[min(li, 1)][:]])
                else:
                    if not no_edge:
                        edge_phase(li, xl[li], xr[li], xoT2, F32)

            # ---------------- MLP head ----------------
            w1t_t = wpool.tile([P, D], F32, tag="w1t")
            nc.sync.dma_start(out=w1t_t[:], in_=w1t[:])
            b1_t = wpool.tile([1, D], F32, tag="b1row")
            nc.sync.dma_start(out=b1_t[:], in_=b1row[:])
            w2t_t = wpool.tile([P, DOUT], F32, tag="w2t")
            nc.sync.dma_start(out=w2t_t[:], in_=w2t[:])
            b2_t = wpool.tile([1, DOUT], F32, tag="b2row")
            nc.sync.dma_start(out=b2_t[:], in_=b2row[:])
            for jj in range(NST):
                x3_t = niop.tile([P, P], F32, tag="x3t")
                nc.sync.dma_start(out=x3_t[:], in_=xoT2[:, jj * P:(jj + 1) * P])
                hps = psA.tile([P, P], F32, tag="psA")
                # hT[d, n] = sum_k W1[d,k] x3[n,k]
                nc.tensor.matmul(out=hps[:], lhsT=w1t_t[:], rhs=x3_t[:],
                                 start=True, stop=False)
                nc.tensor.matmul(out=hps[:], lhsT=b1_t[:], rhs=onesrow0_t[:],
                                 start=False, stop=True)
                h_t = niop.tile([P, P], F32, tag="ht")
                nc.scalar.activation(h_t[:], hps[:], AF.Copy)
                yps = psA.tile([DOUT, P], F32, tag="psA")
                nc.tensor.matmul(out=yps[:], lhsT=w2t_t[:], rhs=h_t[:],
                                 start=True, stop=False)
                nc.tensor.matmul(out=yps[:], lhsT=b2_t[:], rhs=onesrow0_t[:],
                                 start=False, stop=True)
                y_t = niop.tile([DOUT, P], F32, tag="yt")
                nc.scalar.activation(y_t[:], yps[:], AF.Copy)
                nc.sync.dma_start(out=yT[:, jj * P:(jj + 1) * P], in_=y_t[:])

    nc.compile()
    return nc


def _make_in_maps(inputs, ep):
    x = np.asarray(inputs["x"], np.float32)
    Wl = np.asarray(inputs["Wl"], np.float32)
    bl = np.asarray(inputs["bl"], np.float32)
    Wr = np.asarray(inputs["Wr"], np.float32)
    br = np.asarray(inputs["br"], np.float32)
    att = np.asarray(inputs["att"], np.float32)
    bias = np.asarray(inputs["bias"], np.float32)
    W1 = np.asarray(inputs["W1"], np.float32)
    b1 = np.asarray(inputs["b1"], np.float32)
    W2 = np.asarray(inputs["W2"], np.float32)
    b2 = np.asarray(inputs["b2"], np.float32)

    xTp = np.zeros((P, NP_), np.float32)
    xTp[:, :N] = x.T
    common = {
        "xT0": xTp,
        "wlt0": Wl[0].T.copy(),
        "wrt0": Wr[0].T.copy(),
        "wltb": np.stack([Wl[1].T, Wl[2].T]).astype(XDT_NP),
        "wrtb": np.stack([Wr[1].T, Wr[2].T]).astype(XDT_NP),
        "blrow0": bl[0][None, :].copy(),
        "brrow0": br[0][None, :].copy(),
        "blrowb": np.stack([bl[1][None, :], bl[2][None, :]]).astype(XDT_NP),
        "brrowb": np.stack([br[1][None, :], br[2][None, :]]).astype(XDT_NP),
        "att_bc": np.repeat(att[:, None, :], P, axis=1).astype(EDT_NP),
        "biascol": bias[:, :, None].copy(),
        "w1t": W1.T.copy(),
        "b1row": b1[None, :].copy(),
        "w2t": W2.T.copy(),
        "b2row": b2[None, :].copy(),
        "iota_in": np.tile(np.arange(P, dtype=np.float32), (P, 1)).astype(EDT_NP),
        "ident_in": np.eye(P, dtype=np.float32),
        "onescol_in": np.ones((P, 1), EDT_NP),
        "onesrow0": np.ones((1, P), np.float32),
        "onesrowb": np.ones((1, P), XDT_NP),
        "onesrowe": np.ones((1, P), EDT_NP),
        "epsone": np.full((1, 1), 1e-30, EDT_NP),
    }
    in_maps = []
    for c in range(NCORE):
        m = dict(common)
        m["xT0own"] = xTp[:, c * PC:(c + 1) * PC].copy()
        m["srcidx"] = ep["srcidx"][c]
        m["xridx"] = ep["xridx"][c]
        m["dstloc"] = ep["dstloc"][c].astype(EDT_NP)
        in_maps.append(m)
    return in_maps


def _get_compiled(edge_index):
    key = hashlib.md5(np.asarray(edge_index).tobytes()).hexdigest()
    if key not in _CACHE:
        ep = _prep_edges(edge_index)
        nc = _build_program(ep["T"])
        _CACHE[key] = (nc, ep)
    return _CACHE[key]


def _assemble(results):
    y = np.zeros((N, DOUT), np.float32)
    for c in range(NCORE):
        sl = results[c]["yT"].T  # [PC, DOUT]
        lo = c * PC
        hi = min((c + 1) * PC, N)
        if lo < N:
            y[lo:hi] = sl[: hi - lo]
    return y


def kernel(**inputs):
    nc, ep = _get_compiled(inputs["edge_index"])
    in_maps = _make_in_maps(inputs, ep)
    res = run_bass_kernel_spmd(nc, in_maps, core_ids=list(range(NCORE)))
    return _assemble(res.results)



# revision 2
# speedup vs baseline: 1.0522x; 1.0522x over previous
"""GATv2 stack (3 layers + MLP head) on 8 Trainium2 NeuronCores.

v2: minimizes host->device transfer (the dominant cost over the axon
tunnel). x arrives sharded per core (fp16, feature-major); each core
computes lin_l/lin_r for its own nodes only and the xl table is
AllGather'ed on device. Edge indices arrive compact (uint16/int8) and are
widened on device.

Self-contained: takes full inputs, shards internally (dst-range node
partition), runs one SPMD Bass kernel on cores 0-7, returns full output.
"""
import sys

sys.path.insert(0, "/opt/trn_rl_repo")

import hashlib
import os as _os

import numpy as np
import ml_dtypes

import concourse.bass as bass
import concourse.tile as tile
from concourse import bacc, mybir
from concourse.bass_utils import run_bass_kernel_spmd

AF = mybir.ActivationFunctionType
ALU = mybir.AluOpType
F32 = mybir.dt.float32
BF16 = mybir.dt.bfloat16
FP16 = mybir.dt.float16
U16 = mybir.dt.uint16
I8 = mybir.dt.int8
I32 = mybir.dt.int32

P = 128
D = 128
DOUT = 64
N = 50000
NP_ = 50176            # padded nodes: 8 * 49 * 128
PC = 6272              # nodes per core
NST = 49               # super-tiles (128-dst blocks) per core
NCORE = 8
NEG = 0.2
NLAYER = 3
SLAB = 7 * P           # 896 nodes per input slab DMA

# edge-stage dtype (validated in baseline): bf16
EDT = BF16
EDT_NP = ml_dtypes.bfloat16
# node-matmul dtype for layers 1,2
XDT = BF16
XDT_NP = ml_dtypes.bfloat16
# x input / layer-0 dtype: fp16 keeps ~f32 accuracy at half the bytes
if _os.environ.get("GAT_XIN", "fp16") == "f32":
    XIN, XIN_NP = F32, np.float32
else:
    XIN, XIN_NP = FP16, np.float16
# output dtype over the wire
if _os.environ.get("GAT_YDT", "fp16") == "f32":
    YDT, YDT_NP = F32, np.float32
else:
    YDT, YDT_NP = FP16, np.float16

_CACHE = {}


def _prep_edges(edge_index):
    src = np.asarray(edge_index[0], dtype=np.int64)
    dst = np.asarray(edge_index[1], dtype=np.int64)
    core = dst // PC
    stl = (dst % PC) // P
    key = core * NST + stl
    order = np.argsort(key, kind="stable")
    src_s, dst_s, key_s = src[order], dst[order], key[order]
    counts = np.bincount(key_s, minlength=NCORE * NST).reshape(NCORE, NST)
    starts = np.zeros(NCORE * NST + 1, np.int64)
    np.cumsum(counts.ravel(), out=starts[1:])

    T = np.ceil(counts.max(axis=0) / P).astype(np.int64)   # [NST]
    T = np.maximum(T, 1)
    CT = int(T.sum())

    srcidx = np.zeros((NCORE, CT * P), np.int64)
    xridx = np.zeros((NCORE, CT * P), np.int64)
    dstloc = np.full((NCORE, CT * P), -1, np.int64)
    off_t = np.concatenate([[0], np.cumsum(T)]) * P

    for c in range(NCORE):
        for s in range(NST):
            k = c * NST + s
            sl = slice(starts[k], starts[k + 1])
            n = starts[k + 1] - starts[k]
            base = off_t[s]
            srcidx[c, base:base + n] = src_s[sl]
            xridx[c, base:base + n] = dst_s[sl] - c * PC
            dstloc[c, base:base + n] = dst_s[sl] % P

    def pack(arr, dt):
        # edge slot i -> [i % P, off + i // P]
        return np.stack([arr[c].reshape(-1, P).T.copy().astype(dt)
                         for c in range(NCORE)])

    return {
        "T": T,
        "srcidx": pack(srcidx, np.uint16),   # [NCORE, 128, CT]
        "dstloc": pack(dstloc, np.int8),
    }


def _build_program(T):
    nc = bacc.Bacc("TRN2", target_bir_lowering=False, debug=False,
                   enable_asserts=True, num_devices=NCORE)
    CT = int(T.sum())

    dram = lambda n, s, d, **kw: nc.dram_tensor(n, s, d, **kw).ap()
    # ---- external inputs ----
    xTown = dram("xTown", [P, PC], XIN, kind="ExternalInput")
    e_sidx = dram("sidx", [P, CT], U16, kind="ExternalInput")
    e_dloc = dram("dloc", [P, CT], I8, kind="ExternalInput")
    wlt0 = dram("wlt0", [P, D], XIN, kind="ExternalInput")
    wrt0 = dram("wrt0", [P, D], XIN, kind="ExternalInput")
    blrow0 = dram("blrow0", [1, D], XIN, kind="ExternalInput")
    brrow0 = dram("brrow0", [1, D], XIN, kind="ExternalInput")
    wltb = dram("wltb", [2, P, D], XDT, kind="ExternalInput")
    wrtb = dram("wrtb", [2, P, D], XDT, kind="ExternalInput")
    blrowb = dram("blrowb", [2, 1, D], XDT, kind="ExternalInput")
    brrowb = dram("brrowb", [2, 1, D], XDT, kind="ExternalInput")
    attrow = dram("attrow", [NLAYER, 1, D], EDT, kind="ExternalInput")
    biascol = dram("biascol", [NLAYER, P, 1], F32, kind="ExternalInput")
    wft = dram("wft", [P, DOUT], XIN, kind="ExternalInput")
    bfrow = dram("bfrow", [1, DOUT], XIN, kind="ExternalInput")
    onescol_in = dram("onescol_in", [P, 1], EDT, kind="ExternalInput")
    onesrow_x_in = dram("onesrow_x", [1, P], XIN, kind="ExternalInput")
    onesrow_b_in = dram("onesrow_b", [1, P], XDT, kind="ExternalInput")
    onesrow_e_in = dram("onesrow_e", [1, P], EDT, kind="ExternalInput")
    epsone_in = dram("epsone", [1, 1], EDT, kind="ExternalInput")

    # ---- internal DRAM ----
    xl_own = [dram(f"xlown{i}", [PC, D], EDT) for i in range(NLAYER)]
    xr = [dram(f"xr{i}", [PC, D], EDT) for i in range(NLAYER)]
    xlg = [dram(f"xlg{i}", [NP_, D], EDT, addr_space="Shared")
           for i in range(NLAYER)]
    xoTb = [dram(f"xoT{i}b", [P, PC], XDT) for i in range(2)]
    xoT2 = dram("xoT2", [P, PC], XIN)
    yT = dram("yT", [DOUT, PC], YDT, kind="ExternalOutput")

    with tile.TileContext(nc) as tc:
        with (
            tc.tile_pool(name="const", bufs=1) as cpool,
            tc.tile_pool(name="wts", bufs=1) as wpool,
            tc.tile_pool(name="slab", bufs=3) as slabp,
            tc.tile_pool(name="nodeio", bufs=4) as niop,
            tc.tile_pool(name="idx", bufs=3) as idxp,
            tc.tile_pool(name="gath", bufs=2) as gathp,
            tc.tile_pool(name="edge", bufs=4) as edgep,
            tc.tile_pool(name="stt", bufs=3) as sttp,
            tc.tile_pool(name="epi", bufs=3) as epip,
            tc.tile_pool(name="psA", bufs=2, space="PSUM") as psA,
            tc.tile_pool(name="psE", bufs=2, space="PSUM") as psE,
            tc.tile_pool(name="psT", bufs=2, space="PSUM") as psT,
        ):
            # constants (iota/ident generated on device)
            iota_t = cpool.tile([P, P], EDT)
            nc.gpsimd.iota(iota_t[:], [[1, P]], channel_multiplier=0,
                           allow_small_or_imprecise_dtypes=True)
            iotaF = cpool.tile([P, P], F32)
            nc.gpsimd.iota(iotaF[:], [[1, P]], channel_multiplier=0,
                           allow_small_or_imprecise_dtypes=True)
            iotaP = cpool.tile([P, P], F32)
            nc.gpsimd.iota(iotaP[:], [[0, P]], channel_multiplier=1,
                           allow_small_or_imprecise_dtypes=True)
            ident_t = cpool.tile([P, P], F32)
            nc.vector.scalar_tensor_tensor(
                out=ident_t[:], in0=iotaF[:], scalar=1.0, in1=iotaP[:],
                op0=ALU.mult, op1=ALU.is_equal)
            stepcol = cpool.tile([P, NST], F32)
            nc.gpsimd.iota(stepcol[:], [[P, NST]], channel_multiplier=0,
                           allow_small_or_imprecise_dtypes=True)
            onescol_t = cpool.tile([P, 1], EDT)
            nc.sync.dma_start(out=onescol_t[:], in_=onescol_in[:])
            onesrow_x_t = cpool.tile([1, P], XIN)
            nc.sync.dma_start(out=onesrow_x_t[:], in_=onesrow_x_in[:])
            onesrow_b_t = cpool.tile([1, P], XDT)
            nc.sync.dma_start(out=onesrow_b_t[:], in_=onesrow_b_in[:])
            onesrow_e_t = cpool.tile([1, P], EDT)
            nc.sync.dma_start(out=onesrow_e_t[:], in_=onesrow_e_in[:])
            epsone_t = cpool.tile([1, 1], EDT)
            nc.sync.dma_start(out=epsone_t[:], in_=epsone_in[:])

            off_t = np.concatenate([[0], np.cumsum(T)]).astype(int)

            def node_matmul_phase(src_ap, dt_mm, wl_ap, wr_ap, bl_ap, br_ap,
                                  ones_t, xl_out, xr_out, li):
                """lin_l and lin_r for this core's own PC nodes."""
                wl_t = wpool.tile([P, D], dt_mm, tag=f"wl{li}")
                nc.sync.dma_start(out=wl_t[:], in_=wl_ap)
                wr_t = wpool.tile([P, D], dt_mm, tag=f"wr{li}")
                nc.sync.dma_start(out=wr_t[:], in_=wr_ap)
                bl_t = wpool.tile([1, D], dt_mm, tag=f"bl{li}")
                nc.sync.dma_start(out=bl_t[:], in_=bl_ap)
                br_t = wpool.tile([1, D], dt_mm, tag=f"br{li}")
                nc.sync.dma_start(out=br_t[:], in_=br_ap)

                for sl in range(7):
                    st = slabp.tile([P, SLAB], dt_mm, tag="xslab")
                    nc.sync.dma_start(
                        out=st[:], in_=src_ap[:, sl * SLAB:(sl + 1) * SLAB])
                    for t in range(7):
                        j = sl * 7 + t
                        psl = psA.tile([P, D], F32, tag="psA")
                        nc.tensor.matmul(out=psl[:], lhsT=st[:, t * P:(t + 1) * P],
                                         rhs=wl_t[:], start=True, stop=False)
                        nc.tensor.matmul(out=psl[:], lhsT=ones_t[:], rhs=bl_t[:],
                                         start=False, stop=True)
                        ol = niop.tile([P, D], EDT, tag="xlout")
                        nc.scalar.activation(ol[:], psl[:], AF.Copy)
                        nc.sync.dma_start(out=xl_out[j * P:(j + 1) * P, :], in_=ol[:])
                        psr = psA.tile([P, D], F32, tag="psA")
                        nc.tensor.matmul(out=psr[:], lhsT=st[:, t * P:(t + 1) * P],
                                         rhs=wr_t[:], start=True, stop=False)
                        nc.tensor.matmul(out=psr[:], lhsT=ones_t[:], rhs=br_t[:],
                                         start=False, stop=True)
                        orr = niop.tile([P, D], EDT, tag="xlout")
                        nc.scalar.activation(orr[:], psr[:], AF.Copy)
                        nc.sync.dma_start(out=xr_out[j * P:(j + 1) * P, :], in_=orr[:])

            def edge_phase(li, xl_ap, xr_ap, out_own_ap, out_dt):
                # broadcast att row -> [P, D] via ones-column matmul
                att_row_t = wpool.tile([1, D], EDT, tag=f"attr{li}")
                nc.sync.dma_start(out=att_row_t[:], in_=attrow[li])
                att_ps = psA.tile([P, D], F32, tag="psA")
                nc.tensor.matmul(out=att_ps[:], lhsT=onesrow_e_t[:],
                                 rhs=att_row_t[:], start=True, stop=True)
                att_t = wpool.tile([P, D], EDT, tag=f"att{li}")
                nc.scalar.activation(att_t[:], att_ps[:], AF.Copy)
                bias_t = wpool.tile([P, 1], F32, tag=f"bias{li}")
                nc.sync.dma_start(out=bias_t[:], in_=biascol[li])

                for s in range(NST):
                    tt = int(T[s])
                    # index slices for this super-tile (compact dtypes)
                    is16 = idxp.tile([P, tt], U16, tag="is16")
                    nc.sync.dma_start(
                        out=is16[:], in_=e_sidx[:, off_t[s]:off_t[s] + tt])
                    d8 = idxp.tile([P, tt], I8, tag="d8")
                    nc.sync.dma_start(out=d8[:], in_=e_dloc[:, off_t[s]:off_t[s] + tt])
                    dl_t = idxp.tile([P, tt], EDT, tag="dl")
                    nc.vector.tensor_copy(dl_t[:], d8[:])
                    is_off = idxp.tile([P, tt], I32, tag="is32")
                    nc.vector.tensor_copy(is_off[:], is16[:])
                    # local dst row = relu(dloc + s*128); pads (dloc=-1) land
                    # on a valid in-range row and are masked out by selx
                    dl_f = idxp.tile([P, tt], F32, tag="dlf")
                    nc.vector.tensor_copy(dl_f[:], d8[:])
                    ir_off = idxp.tile([P, tt], I32, tag="ir32")
                    nc.scalar.activation(ir_off[:], dl_f[:], AF.Relu,
                                         bias=stepcol[:, s:s + 1])

                    xlbuf = gathp.tile([P, tt, D], EDT, tag="xlbuf")
                    xrbuf = gathp.tile([P, tt, D], EDT, tag="xrbuf")
                    for t in range(tt):
                        nc.gpsimd.indirect_dma_start(
                            out=xlbuf[:, t, :], out_offset=None, in_=xl_ap[:],
                            in_offset=bass.IndirectOffsetOnAxis(
                                ap=is_off[:, t:t + 1], axis=0))
                        nc.gpsimd.indirect_dma_start(
                            out=xrbuf[:, t, :], out_offset=None, in_=xr_ap[:],
                            in_offset=bass.IndirectOffsetOnAxis(
                                ap=ir_off[:, t:t + 1], axis=0))

                    logits_t = edgep.tile([P, tt], F32, tag="logits")
                    for t in range(tt):
                        xlg_ = xlbuf[:, t, :]
                        xrg_ = xrbuf[:, t, :]
                        t1 = sttp.tile([P, D], EDT, tag="t1")
                        nc.vector.tensor_add(t1[:], xlg_, xrg_)
                        lr = sttp.tile([P, D], EDT, tag="lr")
                        nc.vector.scalar_tensor_tensor(
                            out=lr[:], in0=t1[:], scalar=NEG, in1=t1[:],
                            op0=ALU.mult, op1=ALU.max)
                        junk = sttp.tile([P, D], EDT, tag="junk")
                        nc.vector.scalar_tensor_tensor(
                            out=junk[:], in0=lr[:], scalar=1.0, in1=att_t[:],
                            op0=ALU.mult, op1=ALU.mult,
                            accum_out=logits_t[:, t:t + 1])
                    ex_t = edgep.tile([P, tt], EDT, tag="ex")
                    nc.scalar.activation(ex_t[:], logits_t[:], AF.Exp)

                    psf = psE.tile([P, D], F32, tag="psf")
                    psd = psE.tile([P, 1], F32, tag="psd")
                    for t in range(tt):
                        selx = edgep.tile([P, P], EDT, tag="selx")
                        nc.vector.scalar_tensor_tensor(
                            out=selx[:], in0=iota_t[:], scalar=dl_t[:, t:t + 1],
                            in1=ex_t[:, t:t + 1].to_broadcast([P, P]),
                            op0=ALU.is_equal, op1=ALU.mult)
                        nc.tensor.matmul(out=psf[:], lhsT=selx[:],
                                         rhs=xlbuf[:, t, :],
                                         start=(t == 0), stop=(t == tt - 1))
                        nc.tensor.matmul(out=psd[:], lhsT=selx[:],
                                         rhs=onescol_t[:],
                                         start=(t == 0), stop=False)
                    nc.tensor.matmul(out=psd[:], lhsT=onesrow_e_t[:],
                                     rhs=epsone_t[:], start=False, stop=True)
                    # epilogue
                    rec_t = epip.tile([P, 1], F32, tag="rec")
                    nc.vector.reciprocal(rec_t[:], psd[:])
                    outn = epip.tile([P, D], F32, tag="outn")
                    nc.scalar.activation(outn[:], psf[:], AF.Copy,
                                         scale=rec_t[:])
                    tps = psT.tile([P, D], F32, tag="psT")
                    nc.tensor.transpose(out=tps[:], in_=outn[:], identity=ident_t[:])
                    outT = epip.tile([P, D], out_dt, tag="outT")
                    nc.scalar.activation(outT[:], tps[:], AF.Relu, bias=bias_t[:])
                    nc.sync.dma_start(
                        out=out_own_ap[:, s * P:(s + 1) * P], in_=outT[:])

            # ---------------- layers ----------------
            for li in range(NLAYER):
                if li == 0:
                    node_matmul_phase(xTown, XIN, wlt0[:], wrt0[:],
                                      blrow0[:], brrow0[:], onesrow_x_t,
                                      xl_own[0], xr[0], 0)
                else:
                    node_matmul_phase(xoTb[li - 1], XDT,
                                      wltb[li - 1], wrtb[li - 1],
                                      blrowb[li - 1], brrowb[li - 1],
                                      onesrow_b_t, xl_own[li], xr[li], li)
                nc.gpsimd.collective_compute(
                    "AllGather", ALU.bypass,
                    replica_groups=[list(range(NCORE))],
                    ins=[xl_own[li][:]], outs=[xlg[li][:]])
                if li < NLAYER - 1:
                    edge_phase(li, xlg[li], xr[li], xoTb[li], XDT)
                else:
                    edge_phase(li, xlg[li], xr[li], xoT2, XIN)

            # ---------------- MLP head (folded: y = x @ Wf.T + bf) ------
            wft_t = wpool.tile([P, DOUT], XIN, tag="wft")
            nc.sync.dma_start(out=wft_t[:], in_=wft[:])
            bf_t = wpool.tile([1, DOUT], XIN, tag="bfrow")
            nc.sync.dma_start(out=bf_t[:], in_=bfrow[:])
            for jj in range(NST):
                x3_t = niop.tile([P, P], XIN, tag="x3t")
                nc.sync.dma_start(out=x3_t[:], in_=xoT2[:, jj * P:(jj + 1) * P])
                yps = psA.tile([DOUT, P], F32, tag="psA")
                nc.tensor.matmul(out=yps[:], lhsT=wft_t[:], rhs=x3_t[:],
                                 start=True, stop=False)
                nc.tensor.matmul(out=yps[:], lhsT=bf_t[:], rhs=onesrow_x_t[:],
                                 start=False, stop=True)
                y_t = niop.tile([DOUT, P], YDT, tag="yt")
                nc.scalar.activation(y_t[:], yps[:], AF.Copy)
                nc.sync.dma_start(out=yT[:, jj * P:(jj + 1) * P], in_=y_t[:])

    nc.compile()
    return nc


def _make_in_maps(inputs, ep):
    x = np.asarray(inputs["x"], np.float32)
    Wl = np.asarray(inputs["Wl"], np.float32)
    bl = np.asarray(inputs["bl"], np.float32)
    Wr = np.asarray(inputs["Wr"], np.float32)
    br = np.asarray(inputs["br"], np.float32)
    att = np.asarray(inputs["att"], np.float32)
    bias = np.asarray(inputs["bias"], np.float32)
    W1 = np.asarray(inputs["W1"], np.float32)
    b1 = np.asarray(inputs["b1"], np.float32)
    W2 = np.asarray(inputs["W2"], np.float32)
    b2 = np.asarray(inputs["b2"], np.float32)

    xTp = np.zeros((P, NP_), XIN_NP)
    xTp[:, :N] = x.T
    common = {
        "wlt0": Wl[0].T.astype(XIN_NP),
        "wrt0": Wr[0].T.astype(XIN_NP),
        "blrow0": bl[0][None, :].astype(XIN_NP),
        "brrow0": br[0][None, :].astype(XIN_NP),
        "wltb": np.stack([Wl[1].T, Wl[2].T]).astype(XDT_NP),
        "wrtb": np.stack([Wr[1].T, Wr[2].T]).astype(XDT_NP),
        "blrowb": np.stack([bl[1][None, :], bl[2][None, :]]).astype(XDT_NP),
        "brrowb": np.stack([br[1][None, :], br[2][None, :]]).astype(XDT_NP),
        "attrow": att[:, None, :].astype(EDT_NP),
        "biascol": bias[:, :, None].copy(),
        "wft": (W2 @ W1).T.astype(XIN_NP),
        "bfrow": (W2 @ b1 + b2)[None, :].astype(XIN_NP),
        "onescol_in": np.ones((P, 1), EDT_NP),
        "onesrow_x": np.ones((1, P), XIN_NP),
        "onesrow_b": np.ones((1, P), XDT_NP),
        "onesrow_e": np.ones((1, P), EDT_NP),
        "epsone": np.full((1, 1), 1e-30, EDT_NP),
    }
    in_maps = []
    for c in range(NCORE):
        m = dict(common)
        m["xTown"] = xTp[:, c * PC:(c + 1) * PC].copy()
        m["sidx"] = ep["srcidx"][c]
        m["dloc"] = ep["dstloc"][c]
        in_maps.append(m)
    return in_maps


def _get_compiled(edge_index):
    key = hashlib.md5(np.asarray(edge_index).tobytes()).hexdigest()
    if key not in _CACHE:
        ep = _prep_edges(edge_index)
        nc = _build_program(ep["T"])
        _CACHE[key] = (nc, ep)
    return _CACHE[key]


def _assemble(results):
    y = np.zeros((N, DOUT), np.float32)
    for c in range(NCORE):
        sl = results[c]["yT"].T.astype(np.float32)  # [PC, DOUT]
        lo = c * PC
        hi = min((c + 1) * PC, N)
        if lo < N:
            y[lo:hi] = sl[: hi - lo]
    return y


def kernel(**inputs):
    nc, ep = _get_compiled(inputs["edge_index"])
    in_maps = _make_in_maps(inputs, ep)
    res = run_bass_kernel_spmd(nc, in_maps, core_ids=list(range(NCORE)))
    return _assemble(res.results)


# revision 3
# speedup vs baseline: 1.3615x; 1.2940x over previous
"""GATv2 stack (3 layers + MLP head) on 8 Trainium2 NeuronCores.

v5: all per-core inputs are packed into ONE uint8 blob (the axon tunnel
charges ~3.7 ms per transferred array on top of ~280 MB/s, so 20 arrays
-> 1 saves ~70 ms per call). On device the blob is viewed through
bitcast/rearrange APs at fixed offsets.

Sharding: nodes partitioned across cores by dst range; x arrives sharded
(fp16, feature-major); each core computes lin_l/lin_r for its own nodes
and the xl table is AllGather'ed on device. Edge indices arrive compact
(uint16/int8) and are widened on device. The linear MLP head is folded
into a single matmul on the host.

Self-contained: takes full inputs, returns the full output.
"""
import sys

sys.path.insert(0, "/opt/trn_rl_repo")

import hashlib
import os as _os

import numpy as np
import ml_dtypes

import concourse.bass as bass
import concourse.tile as tile
from concourse import bacc, mybir
from concourse.bass_utils import run_bass_kernel_spmd

AF = mybir.ActivationFunctionType
ALU = mybir.AluOpType
F32 = mybir.dt.float32
BF16 = mybir.dt.bfloat16
FP16 = mybir.dt.float16
U16 = mybir.dt.uint16
U8 = mybir.dt.uint8
I8 = mybir.dt.int8
I32 = mybir.dt.int32

P = 128
D = 128
DOUT = 64
N = 50000
NP_ = 50176            # padded nodes: 8 * 49 * 128
PC = 6272              # nodes per core
NST = 49               # super-tiles (128-dst blocks) per core
NCORE = 8
NEG = 0.2
NLAYER = 3
SLAB = 7 * P           # 896 nodes per input slab DMA

EDT = BF16             # edge-stage dtype
EDT_NP = ml_dtypes.bfloat16
XDT = BF16             # node-matmul dtype, layers 1-2
XDT_NP = ml_dtypes.bfloat16
XIN, XIN_NP = FP16, np.float16    # x input / layer-0 / MLP dtype
YDT, YDT_NP = FP16, np.float16    # output dtype over the wire

_CACHE = {}

_NPDT = {FP16: np.float16, BF16: ml_dtypes.bfloat16, F32: np.float32,
         U16: np.uint16, I8: np.int8}


def _manifest(CT):
    """Packing order of every per-core input inside the u8 blob."""
    ent = [
        ("xTown", (P, PC), XIN),
        ("sidx", (P, CT), U16),
        ("dloc", (P, CT), I8),
        ("wlt0", (P, D), XIN),
        ("wrt0", (P, D), XIN),
        ("wft", (P, DOUT), XIN),
        ("wltb0", (P, D), XDT),
        ("wltb1", (P, D), XDT),
        ("wrtb0", (P, D), XDT),
        ("wrtb1", (P, D), XDT),
        ("blrow0", (1, D), XIN),
        ("brrow0", (1, D), XIN),
        ("bfrow", (1, DOUT), XIN),
        ("onesrow_x", (1, P), XIN),
        ("blrowb0", (1, D), XDT),
        ("blrowb1", (1, D), XDT),
        ("brrowb0", (1, D), XDT),
        ("brrowb1", (1, D), XDT),
        ("onesrow_b", (1, P), XDT),
        ("attrow0", (1, D), EDT),
        ("attrow1", (1, D), EDT),
        ("attrow2", (1, D), EDT),
        ("onesrow_e", (1, P), EDT),
        ("onescol", (P, 1), EDT),
        ("epsone", (1, 1), EDT),
        ("biascol0", (P, 1), F32),
        ("biascol1", (P, 1), F32),
        ("biascol2", (P, 1), F32),
    ]
    out = {}
    off = 0
    for name, shape, mdt in ent:
        nbytes = shape[0] * shape[1] * mybir.dt.size(mdt)
        out[name] = (off, shape, mdt)
        off += (nbytes + 63) // 64 * 64
    total = (off + 63) // 64 * 64
    return out, total


def _prep_edges(edge_index):
    src = np.asarray(edge_index[0], dtype=np.int64)
    dst = np.asarray(edge_index[1], dtype=np.int64)
    core = dst // PC
    stl = (dst % PC) // P
    key = core * NST + stl
    order = np.argsort(key, kind="stable")
    src_s, dst_s, key_s = src[order], dst[order], key[order]
    counts = np.bincount(key_s, minlength=NCORE * NST).reshape(NCORE, NST)
    starts = np.zeros(NCORE * NST + 1, np.int64)
    np.cumsum(counts.ravel(), out=starts[1:])

    T = np.ceil(counts.max(axis=0) / P).astype(np.int64)   # [NST]
    T = np.maximum(T, 1)
    CT = int(T.sum())

    srcidx = np.zeros((NCORE, CT * P), np.int64)
    dstloc = np.full((NCORE, CT * P), -1, np.int64)
    off_t = np.concatenate([[0], np.cumsum(T)]) * P

    for c in range(NCORE):
        for s in range(NST):
            k = c * NST + s
            sl = slice(starts[k], starts[k + 1])
            n = starts[k + 1] - starts[k]
            base = off_t[s]
            srcidx[c, base:base + n] = src_s[sl]
            dstloc[c, base:base + n] = dst_s[sl] % P

    def pack(arr, dt):
        # edge slot i -> [i % P, off + i // P]
        return np.stack([arr[c].reshape(-1, P).T.copy().astype(dt)
                         for c in range(NCORE)])

    return {
        "T": T,
        "srcidx": pack(srcidx, np.uint16),   # [NCORE, 128, CT]
        "dstloc": pack(dstloc, np.int8),
    }


def _build_program(T):
    nc = bacc.Bacc("TRN2", target_bir_lowering=False, debug=False,
                   enable_asserts=True, num_devices=NCORE)
    CT = int(T.sum())
    man, TOTAL = _manifest(CT)

    dram = lambda n, s, d, **kw: nc.dram_tensor(n, s, d, **kw).ap()
    blob = dram("blob", [1, TOTAL], U8, kind="ExternalInput")

    def V(name):
        off, (a, b), mdt = man[name]
        nb = a * b * mybir.dt.size(mdt)
        return (blob[0:1, off:off + nb].bitcast(mdt)
                .rearrange("o (a b) -> (o a) b", a=a))

    # ---- internal DRAM ----
    xl_own = [dram(f"xlown{i}", [PC, D], EDT) for i in range(NLAYER)]
    xr = [dram(f"xr{i}", [PC, D], EDT) for i in range(NLAYER)]
    xlg = [dram(f"xlg{i}", [NP_, D], EDT, addr_space="Shared")
           for i in range(NLAYER)]
    xoTb = [dram(f"xoT{i}b", [P, PC], XDT) for i in range(2)]
    xoT2 = dram("xoT2", [P, PC], XIN)
    yT = dram("yT", [DOUT, PC], YDT, kind="ExternalOutput")

    e_sidx = V("sidx")
    e_dloc = V("dloc")

    with tile.TileContext(nc) as tc:
        with (
            tc.tile_pool(name="const", bufs=1) as cpool,
            tc.tile_pool(name="wts", bufs=1) as wpool,
            tc.tile_pool(name="slab", bufs=3) as slabp,
            tc.tile_pool(name="nodeio", bufs=4) as niop,
            tc.tile_pool(name="idx", bufs=3) as idxp,
            tc.tile_pool(name="gath", bufs=2) as gathp,
            tc.tile_pool(name="edge", bufs=4) as edgep,
            tc.tile_pool(name="stt", bufs=3) as sttp,
            tc.tile_pool(name="epi", bufs=3) as epip,
            tc.tile_pool(name="psA", bufs=2, space="PSUM") as psA,
            tc.tile_pool(name="psE", bufs=2, space="PSUM") as psE,
            tc.tile_pool(name="psT", bufs=2, space="PSUM") as psT,
        ):
            # constants (iota/ident generated on device)
            iota_t = cpool.tile([P, P], EDT)
            nc.gpsimd.iota(iota_t[:], [[1, P]], channel_multiplier=0,
                           allow_small_or_imprecise_dtypes=True)
            iotaF = cpool.tile([P, P], F32)
            nc.gpsimd.iota(iotaF[:], [[1, P]], channel_multiplier=0,
                           allow_small_or_imprecise_dtypes=True)
            iotaP = cpool.tile([P, P], F32)
            nc.gpsimd.iota(iotaP[:], [[0, P]], channel_multiplier=1,
                           allow_small_or_imprecise_dtypes=True)
            ident_t = cpool.tile([P, P], F32)
            nc.vector.scalar_tensor_tensor(
                out=ident_t[:], in0=iotaF[:], scalar=1.0, in1=iotaP[:],
                op0=ALU.mult, op1=ALU.is_equal)
            stepcol = cpool.tile([P, NST], F32)
            nc.gpsimd.iota(stepcol[:], [[P, NST]], channel_multiplier=0,
                           allow_small_or_imprecise_dtypes=True)
            onescol_t = cpool.tile([P, 1], EDT)
            nc.sync.dma_start(out=onescol_t[:], in_=V("onescol"))
            onesrow_x_t = cpool.tile([1, P], XIN)
            nc.sync.dma_start(out=onesrow_x_t[:], in_=V("onesrow_x"))
            onesrow_b_t = cpool.tile([1, P], XDT)
            nc.sync.dma_start(out=onesrow_b_t[:], in_=V("onesrow_b"))
            onesrow_e_t = cpool.tile([1, P], EDT)
            nc.sync.dma_start(out=onesrow_e_t[:], in_=V("onesrow_e"))
            epsone_t = cpool.tile([1, 1], EDT)
            nc.sync.dma_start(out=epsone_t[:], in_=V("epsone"))

            off_t = np.concatenate([[0], np.cumsum(T)]).astype(int)

            def node_matmul_phase(src_ap, dt_mm, wl_ap, wr_ap, bl_ap, br_ap,
                                  ones_t, xl_out, xr_out, li):
                """lin_l and lin_r for this core's own PC nodes."""
                wl_t = wpool.tile([P, D], dt_mm, tag=f"wl{li}")
                nc.sync.dma_start(out=wl_t[:], in_=wl_ap)
                wr_t = wpool.tile([P, D], dt_mm, tag=f"wr{li}")
                nc.sync.dma_start(out=wr_t[:], in_=wr_ap)
                bl_t = wpool.tile([1, D], dt_mm, tag=f"bl{li}")
                nc.sync.dma_start(out=bl_t[:], in_=bl_ap)
                br_t = wpool.tile([1, D], dt_mm, tag=f"br{li}")
                nc.sync.dma_start(out=br_t[:], in_=br_ap)

                for sl in range(7):
                    st = slabp.tile([P, SLAB], dt_mm, tag="xslab")
                    nc.sync.dma_start(
                        out=st[:], in_=src_ap[:, sl * SLAB:(sl + 1) * SLAB])
                    for t in range(7):
                        j = sl * 7 + t
                        psl = psA.tile([P, D], F32, tag="psA")
                        nc.tensor.matmul(out=psl[:], lhsT=st[:, t * P:(t + 1) * P],
                                         rhs=wl_t[:], start=True, stop=False)
                        nc.tensor.matmul(out=psl[:], lhsT=ones_t[:], rhs=bl_t[:],
                                         start=False, stop=True)
                        ol = niop.tile([P, D], EDT, tag="xlout")
                        nc.scalar.activation(ol[:], psl[:], AF.Copy)
                        nc.sync.dma_start(out=xl_out[j * P:(j + 1) * P, :], in_=ol[:])
                        psr = psA.tile([P, D], F32, tag="psA")
                        nc.tensor.matmul(out=psr[:], lhsT=st[:, t * P:(t + 1) * P],
                                         rhs=wr_t[:], start=True, stop=False)
                        nc.tensor.matmul(out=psr[:], lhsT=ones_t[:], rhs=br_t[:],
                                         start=False, stop=True)
                        orr = niop.tile([P, D], EDT, tag="xlout")
                        nc.scalar.activation(orr[:], psr[:], AF.Copy)
                        nc.sync.dma_start(out=xr_out[j * P:(j + 1) * P, :], in_=orr[:])

            def edge_phase(li, xl_ap, xr_ap, out_own_ap, out_dt):
                # broadcast att row -> [P, D] via ones-column matmul
                att_row_t = wpool.tile([1, D], EDT, tag=f"attr{li}")
                nc.sync.dma_start(out=att_row_t[:], in_=V(f"attrow{li}"))
                att_ps = psA.tile([P, D], F32, tag="psA")
                nc.tensor.matmul(out=att_ps[:], lhsT=onesrow_e_t[:],
                                 rhs=att_row_t[:], start=True, stop=True)
                att_t = wpool.tile([P, D], EDT, tag=f"att{li}")
                nc.scalar.activation(att_t[:], att_ps[:], AF.Copy)
                bias_t = wpool.tile([P, 1], F32, tag=f"bias{li}")
                nc.sync.dma_start(out=bias_t[:], in_=V(f"biascol{li}"))

                for s in range(NST):
                    tt = int(T[s])
                    # index slices for this super-tile (compact dtypes)
                    is16 = idxp.tile([P, tt], U16, tag="is16")
                    nc.sync.dma_start(
                        out=is16[:], in_=e_sidx[:, off_t[s]:off_t[s] + tt])
                    d8 = idxp.tile([P, tt], I8, tag="d8")
                    nc.sync.dma_start(out=d8[:], in_=e_dloc[:, off_t[s]:off_t[s] + tt])
                    dl_t = idxp.tile([P, tt], EDT, tag="dl")
                    nc.vector.tensor_copy(dl_t[:], d8[:])
                    is_off = idxp.tile([P, tt], I32, tag="is32")
                    nc.vector.tensor_copy(is_off[:], is16[:])
                    # local dst row = relu(dloc + s*128); pads (dloc=-1) land
                    # on a valid in-range row and are masked out by selx
                    dl_f = idxp.tile([P, tt], F32, tag="dlf")
                    nc.vector.tensor_copy(dl_f[:], d8[:])
                    ir_off = idxp.tile([P, tt], I32, tag="ir32")
                    nc.scalar.activation(ir_off[:], dl_f[:], AF.Relu,
                                         bias=stepcol[:, s:s + 1])

                    xlbuf = gathp.tile([P, tt, D], EDT, tag="xlbuf")
                    xrbuf = gathp.tile([P, tt, D], EDT, tag="xrbuf")
                    for t in range(tt):
                        nc.gpsimd.indirect_dma_start(
                            out=xlbuf[:, t, :], out_offset=None, in_=xl_ap[:],
                            in_offset=bass.IndirectOffsetOnAxis(
                                ap=is_off[:, t:t + 1], axis=0))
                        nc.gpsimd.indirect_dma_start(
                            out=xrbuf[:, t, :], out_offset=None, in_=xr_ap[:],
                            in_offset=bass.IndirectOffsetOnAxis(
                                ap=ir_off[:, t:t + 1], axis=0))

                    logits_t = edgep.tile([P, tt], F32, tag="logits")
                    for t in range(tt):
                        xlg_ = xlbuf[:, t, :]
                        xrg_ = xrbuf[:, t, :]
                        t1 = sttp.tile([P, D], EDT, tag="t1")
                        nc.vector.tensor_add(t1[:], xlg_, xrg_)
                        lr = sttp.tile([P, D], EDT, tag="lr")
                        nc.vector.scalar_tensor_tensor(
                            out=lr[:], in0=t1[:], scalar=NEG, in1=t1[:],
                            op0=ALU.mult, op1=ALU.max)
                        junk = sttp.tile([P, D], EDT, tag="junk")
                        nc.vector.scalar_tensor_tensor(
                            out=junk[:], in0=lr[:], scalar=1.0, in1=att_t[:],
                            op0=ALU.mult, op1=ALU.mult,
                            accum_out=logits_t[:, t:t + 1])
                    ex_t = edgep.tile([P, tt], EDT, tag="ex")
                    nc.scalar.activation(ex_t[:], logits_t[:], AF.Exp)

                    psf = psE.tile([P, D], F32, tag="psf")
                    psd = psE.tile([P, 1], F32, tag="psd")
                    for t in range(tt):
                        selx = edgep.tile([P, P], EDT, tag="selx")
                        nc.vector.scalar_tensor_tensor(
                            out=selx[:], in0=iota_t[:], scalar=dl_t[:, t:t + 1],
                            in1=ex_t[:, t:t + 1].to_broadcast([P, P]),
                            op0=ALU.is_equal, op1=ALU.mult)
                        nc.tensor.matmul(out=psf[:], lhsT=selx[:],
                                         rhs=xlbuf[:, t, :],
                                         start=(t == 0), stop=(t == tt - 1))
                        nc.tensor.matmul(out=psd[:], lhsT=selx[:],
                                         rhs=onescol_t[:],
                                         start=(t == 0), stop=False)
                    nc.tensor.matmul(out=psd[:], lhsT=onesrow_e_t[:],
                                     rhs=epsone_t[:], start=False, stop=True)
                    # epilogue
                    rec_t = epip.tile([P, 1], F32, tag="rec")
                    nc.vector.reciprocal(rec_t[:], psd[:])
                    outn = epip.tile([P, D], F32, tag="outn")
                    nc.scalar.activation(outn[:], psf[:], AF.Copy,
                                         scale=rec_t[:])
                    tps = psT.tile([P, D], F32, tag="psT")
                    nc.tensor.transpose(out=tps[:], in_=outn[:], identity=ident_t[:])
                    outT = epip.tile([P, D], out_dt, tag="outT")
                    nc.scalar.activation(outT[:], tps[:], AF.Relu, bias=bias_t[:])
                    nc.sync.dma_start(
                        out=out_own_ap[:, s * P:(s + 1) * P], in_=outT[:])

            # ---------------- layers ----------------
            for li in range(NLAYER):
                if li == 0:
                    node_matmul_phase(V("xTown"), XIN, V("wlt0"), V("wrt0"),
                                      V("blrow0"), V("brrow0"), onesrow_x_t,
                                      xl_own[0], xr[0], 0)
                else:
                    node_matmul_phase(xoTb[li - 1], XDT,
                                      V(f"wltb{li - 1}"), V(f"wrtb{li - 1}"),
                                      V(f"blrowb{li - 1}"), V(f"brrowb{li - 1}"),
                                      onesrow_b_t, xl_own[li], xr[li], li)
                nc.gpsimd.collective_compute(
                    "AllGather", ALU.bypass,
                    replica_groups=[list(range(NCORE))],
                    ins=[xl_own[li][:]], outs=[xlg[li][:]])
                if li < NLAYER - 1:
                    edge_phase(li, xlg[li], xr[li], xoTb[li], XDT)
                else:
                    edge_phase(li, xlg[li], xr[li], xoT2, XIN)

            # ---------------- MLP head (folded: y = x @ Wf.T + bf) ------
            wft_t = wpool.tile([P, DOUT], XIN, tag="wft")
            nc.sync.dma_start(out=wft_t[:], in_=V("wft"))
            bf_t = wpool.tile([1, DOUT], XIN, tag="bfrow")
            nc.sync.dma_start(out=bf_t[:], in_=V("bfrow"))
            for jj in range(NST):
                x3_t = niop.tile([P, P], XIN, tag="x3t")
                nc.sync.dma_start(out=x3_t[:], in_=xoT2[:, jj * P:(jj + 1) * P])
                yps = psA.tile([DOUT, P], F32, tag="psA")
                nc.tensor.matmul(out=yps[:], lhsT=wft_t[:], rhs=x3_t[:],
                                 start=True, stop=False)
                nc.tensor.matmul(out=yps[:], lhsT=bf_t[:], rhs=onesrow_x_t[:],
                                 start=False, stop=True)
                y_t = niop.tile([DOUT, P], YDT, tag="yt")
                nc.scalar.activation(y_t[:], yps[:], AF.Copy)
                nc.sync.dma_start(out=yT[:, jj * P:(jj + 1) * P], in_=y_t[:])

    nc.compile()
    return nc


def _make_in_maps(inputs, ep):
    x = np.asarray(inputs["x"], np.float32)
    Wl = np.asarray(inputs["Wl"], np.float32)
    bl = np.asarray(inputs["bl"], np.float32)
    Wr = np.asarray(inputs["Wr"], np.float32)
    br = np.asarray(inputs["br"], np.float32)
    att = np.asarray(inputs["att"], np.float32)
    bias = np.asarray(inputs["bias"], np.float32)
    W1 = np.asarray(inputs["W1"], np.float32)
    b1 = np.asarray(inputs["b1"], np.float32)
    W2 = np.asarray(inputs["W2"], np.float32)
    b2 = np.asarray(inputs["b2"], np.float32)

    CT = int(ep["T"].sum())
    man, TOTAL = _manifest(CT)

    xTp = np.zeros((P, NP_), XIN_NP)
    xTp[:, :N] = x.T
    common = {
        "wlt0": Wl[0].T.astype(XIN_NP),
        "wrt0": Wr[0].T.astype(XIN_NP),
        "wft": (W2 @ W1).T.astype(XIN_NP),
        "wltb0": Wl[1].T.astype(XDT_NP),
        "wltb1": Wl[2].T.astype(XDT_NP),
        "wrtb0": Wr[1].T.astype(XDT_NP),
        "wrtb1": Wr[2].T.astype(XDT_NP),
        "blrow0": bl[0][None, :].astype(XIN_NP),
        "brrow0": br[0][None, :].astype(XIN_NP),
        "bfrow": (W2 @ b1 + b2)[None, :].astype(XIN_NP),
        "onesrow_x": np.ones((1, P), XIN_NP),
        "blrowb0": bl[1][None, :].astype(XDT_NP),
        "blrowb1": bl[2][None, :].astype(XDT_NP),
        "brrowb0": br[1][None, :].astype(XDT_NP),
        "brrowb1": br[2][None, :].astype(XDT_NP),
        "onesrow_b": np.ones((1, P), XDT_NP),
        "attrow0": att[0][None, :].astype(EDT_NP),
        "attrow1": att[1][None, :].astype(EDT_NP),
        "attrow2": att[2][None, :].astype(EDT_NP),
        "onesrow_e": np.ones((1, P), EDT_NP),
        "onescol": np.ones((P, 1), EDT_NP),
        "epsone": np.full((1, 1), 1e-30, EDT_NP),
        "biascol0": bias[0][:, None].astype(np.float32),
        "biascol1": bias[1][:, None].astype(np.float32),
        "biascol2": bias[2][:, None].astype(np.float32),
    }

    in_maps = []
    for c in range(NCORE):
        vals = dict(common)
        vals["xTown"] = xTp[:, c * PC:(c + 1) * PC]
        vals["sidx"] = ep["srcidx"][c]
        vals["dloc"] = ep["dstloc"][c]
        blob = np.zeros((1, TOTAL), np.uint8)
        for name, (off, shape, mdt) in man.items():
            a = np.ascontiguousarray(vals[name], dtype=_NPDT[mdt])
            assert a.shape == shape, (name, a.shape, shape)
            raw = np.frombuffer(a.tobytes(), np.uint8)
            blob[0, off:off + raw.size] = raw
        in_maps.append({"blob": blob})
    return in_maps


def _get_compiled(edge_index):
    key = hashlib.md5(np.asarray(edge_index).tobytes()).hexdigest()
    if key not in _CACHE:
        ep = _prep_edges(edge_index)
        nc = _build_program(ep["T"])
        _CACHE[key] = (nc, ep)
    return _CACHE[key]


def _assemble(results):
    y = np.zeros((N, DOUT), np.float32)
    for c in range(NCORE):
        sl = results[c]["yT"].T.astype(np.float32)  # [PC, DOUT]
        lo = c * PC
        hi = min((c + 1) * PC, N)
        if lo < N:
            y[lo:hi] = sl[: hi - lo]
    return y


def kernel(**inputs):
    nc, ep = _get_compiled(inputs["edge_index"])
    in_maps = _make_in_maps(inputs, ep)
    res = run_bass_kernel_spmd(nc, in_maps, core_ids=list(range(NCORE)))
    return _assemble(res.results)


# revision 4
# speedup vs baseline: 1.5158x; 1.1133x over previous
"""GATv2 stack (3 layers + MLP head) on 8 Trainium2 NeuronCores.

v5: all per-core inputs are packed into ONE uint8 blob (the axon tunnel
charges ~3.7 ms per transferred array on top of ~280 MB/s, so 20 arrays
-> 1 saves ~70 ms per call). On device the blob is viewed through
bitcast/rearrange APs at fixed offsets.

Sharding: nodes partitioned across cores by dst range; x arrives sharded
(fp16, feature-major); each core computes lin_l/lin_r for its own nodes
and the xl table is AllGather'ed on device. Edge indices arrive compact
(uint16/int8) and are widened on device. The linear MLP head is folded
into a single matmul on the host.

Self-contained: takes full inputs, returns the full output.
"""
import sys

sys.path.insert(0, "/opt/trn_rl_repo")

import hashlib
import os as _os

import numpy as np
import ml_dtypes

import concourse.bass as bass
import concourse.tile as tile
from concourse import bacc, mybir
from concourse.bass_utils import run_bass_kernel_spmd

AF = mybir.ActivationFunctionType
ALU = mybir.AluOpType
F32 = mybir.dt.float32
BF16 = mybir.dt.bfloat16
FP16 = mybir.dt.float16
FP8 = mybir.dt.float8e4          # e4m3
U16 = mybir.dt.uint16
U8 = mybir.dt.uint8
I8 = mybir.dt.int8
I32 = mybir.dt.int32

P = 128
D = 128
DOUT = 64
N = 50000
NP_ = 50176            # padded nodes: 8 * 49 * 128
PC = 6272              # nodes per core
NST = 49               # super-tiles (128-dst blocks) per core
NCORE = 8
NEG = 0.2
NLAYER = 3
SLAB = 7 * P           # 896 nodes per input slab DMA

EDT = BF16             # edge-stage dtype
EDT_NP = ml_dtypes.bfloat16
XDT = BF16             # node-matmul dtype, layers 1-2
XDT_NP = ml_dtypes.bfloat16
XIN, XIN_NP = FP16, np.float16    # x input / layer-0 / MLP dtype
YDT, YDT_NP = FP16, np.float16    # output dtype over the wire

_CACHE = {}

_NPDT = {FP16: np.float16, BF16: ml_dtypes.bfloat16, F32: np.float32,
         U16: np.uint16, I8: np.int8, FP8: ml_dtypes.float8_e4m3fn}


def _manifest(CT):
    """Packing order of every per-core input inside the u8 blob."""
    ent = [
        ("xTown", (P, PC), FP8),
        ("sidx", (P, CT), U16),
        ("dloc", (P, CT), I8),
        ("wlt0", (P, D), XIN),
        ("wrt0", (P, D), XIN),
        ("wft", (P, DOUT), XIN),
        ("wltb0", (P, D), XDT),
        ("wltb1", (P, D), XDT),
        ("wrtb0", (P, D), XDT),
        ("wrtb1", (P, D), XDT),
        ("blrow0", (1, D), XIN),
        ("brrow0", (1, D), XIN),
        ("bfrow", (1, DOUT), XIN),
        ("onesrow_x", (1, P), XIN),
        ("blrowb0", (1, D), XDT),
        ("blrowb1", (1, D), XDT),
        ("brrowb0", (1, D), XDT),
        ("brrowb1", (1, D), XDT),
        ("onesrow_b", (1, P), XDT),
        ("attrow0", (1, D), EDT),
        ("attrow1", (1, D), EDT),
        ("attrow2", (1, D), EDT),
        ("onesrow_e", (1, P), EDT),
        ("onescol", (P, 1), EDT),
        ("epsone", (1, 1), EDT),
        ("biascol0", (P, 1), F32),
        ("biascol1", (P, 1), F32),
        ("biascol2", (P, 1), F32),
    ]
    out = {}
    off = 0
    for name, shape, mdt in ent:
        nbytes = shape[0] * shape[1] * mybir.dt.size(mdt)
        out[name] = (off, shape, mdt)
        off += (nbytes + 63) // 64 * 64
    total = (off + 63) // 64 * 64
    return out, total


def _prep_edges(edge_index):
    src = np.asarray(edge_index[0], dtype=np.int64)
    dst = np.asarray(edge_index[1], dtype=np.int64)
    core = dst // PC
    stl = (dst % PC) // P
    key = core * NST + stl
    order = np.argsort(key, kind="stable")
    src_s, dst_s, key_s = src[order], dst[order], key[order]
    counts = np.bincount(key_s, minlength=NCORE * NST).reshape(NCORE, NST)
    starts = np.zeros(NCORE * NST + 1, np.int64)
    np.cumsum(counts.ravel(), out=starts[1:])

    T = np.ceil(counts.max(axis=0) / P).astype(np.int64)   # [NST]
    T = np.maximum(T, 1)
    CT = int(T.sum())

    srcidx = np.zeros((NCORE, CT * P), np.int64)
    dstloc = np.full((NCORE, CT * P), -1, np.int64)
    off_t = np.concatenate([[0], np.cumsum(T)]) * P

    for c in range(NCORE):
        for s in range(NST):
            k = c * NST + s
            sl = slice(starts[k], starts[k + 1])
            n = starts[k + 1] - starts[k]
            base = off_t[s]
            srcidx[c, base:base + n] = src_s[sl]
            dstloc[c, base:base + n] = dst_s[sl] % P

    def pack(arr, dt):
        # edge slot i -> [i % P, off + i // P]
        return np.stack([arr[c].reshape(-1, P).T.copy().astype(dt)
                         for c in range(NCORE)])

    return {
        "T": T,
        "srcidx": pack(srcidx, np.uint16),   # [NCORE, 128, CT]
        "dstloc": pack(dstloc, np.int8),
    }


def _build_program(T):
    nc = bacc.Bacc("TRN2", target_bir_lowering=False, debug=False,
                   enable_asserts=True, num_devices=NCORE)
    CT = int(T.sum())
    man, TOTAL = _manifest(CT)

    dram = lambda n, s, d, **kw: nc.dram_tensor(n, s, d, **kw).ap()
    # f32-typed blob: the tunnel moves f32 arrays faster than uint8 ones
    blob = dram("blob", [1, TOTAL // 4], F32, kind="ExternalInput")

    def V(name):
        off, (a, b), mdt = man[name]
        sz = mybir.dt.size(mdt)
        nb = a * b * sz
        o4, n4 = off // 4, (nb + 3) // 4
        v = blob[0:1, o4:o4 + n4].bitcast(mdt)
        if a == 1:
            return v[:, :b]
        assert nb % 4 == 0, (name, nb)
        return v.rearrange("o (a b) -> (o a) b", a=a)

    # ---- internal DRAM ----
    xl_own = [dram(f"xlown{i}", [PC, D], EDT) for i in range(NLAYER)]
    xr = [dram(f"xr{i}", [PC, D], EDT) for i in range(NLAYER)]
    xlg = [dram(f"xlg{i}", [NP_, D], EDT, addr_space="Shared")
           for i in range(NLAYER)]
    xoTb = [dram(f"xoT{i}b", [P, PC], XDT) for i in range(2)]
    xoT2 = dram("xoT2", [P, PC], XIN)
    yT = dram("yT", [DOUT, PC], YDT, kind="ExternalOutput")

    e_sidx = V("sidx")
    e_dloc = V("dloc")

    with tile.TileContext(nc) as tc:
        with (
            tc.tile_pool(name="const", bufs=1) as cpool,
            tc.tile_pool(name="wts", bufs=1) as wpool,
            tc.tile_pool(name="slab", bufs=3) as slabp,
            tc.tile_pool(name="nodeio", bufs=4) as niop,
            tc.tile_pool(name="idx", bufs=3) as idxp,
            tc.tile_pool(name="gath", bufs=2) as gathp,
            tc.tile_pool(name="edge", bufs=4) as edgep,
            tc.tile_pool(name="stt", bufs=3) as sttp,
            tc.tile_pool(name="epi", bufs=3) as epip,
            tc.tile_pool(name="psA", bufs=2, space="PSUM") as psA,
            tc.tile_pool(name="psE", bufs=2, space="PSUM") as psE,
            tc.tile_pool(name="psT", bufs=2, space="PSUM") as psT,
        ):
            # constants (iota/ident generated on device)
            iota_t = cpool.tile([P, P], EDT)
            nc.gpsimd.iota(iota_t[:], [[1, P]], channel_multiplier=0,
                           allow_small_or_imprecise_dtypes=True)
            iotaF = cpool.tile([P, P], F32)
            nc.gpsimd.iota(iotaF[:], [[1, P]], channel_multiplier=0,
                           allow_small_or_imprecise_dtypes=True)
            iotaP = cpool.tile([P, P], F32)
            nc.gpsimd.iota(iotaP[:], [[0, P]], channel_multiplier=1,
                           allow_small_or_imprecise_dtypes=True)
            ident_t = cpool.tile([P, P], F32)
            nc.vector.scalar_tensor_tensor(
                out=ident_t[:], in0=iotaF[:], scalar=1.0, in1=iotaP[:],
                op0=ALU.mult, op1=ALU.is_equal)
            stepcol = cpool.tile([P, NST], F32)
            nc.gpsimd.iota(stepcol[:], [[P, NST]], channel_multiplier=0,
                           allow_small_or_imprecise_dtypes=True)
            onescol_t = cpool.tile([P, 1], EDT)
            nc.sync.dma_start(out=onescol_t[:], in_=V("onescol"))
            onesrow_x_t = cpool.tile([1, P], XIN)
            nc.sync.dma_start(out=onesrow_x_t[:], in_=V("onesrow_x"))
            onesrow_b_t = cpool.tile([1, P], XDT)
            nc.sync.dma_start(out=onesrow_b_t[:], in_=V("onesrow_b"))
            onesrow_e_t = cpool.tile([1, P], EDT)
            nc.sync.dma_start(out=onesrow_e_t[:], in_=V("onesrow_e"))
            epsone_t = cpool.tile([1, 1], EDT)
            nc.sync.dma_start(out=epsone_t[:], in_=V("epsone"))

            off_t = np.concatenate([[0], np.cumsum(T)]).astype(int)

            def node_matmul_phase(src_ap, dt_mm, wl_ap, wr_ap, bl_ap, br_ap,
                                  ones_t, xl_out, xr_out, li):
                """lin_l and lin_r for this core's own PC nodes."""
                wl_t = wpool.tile([P, D], dt_mm, tag=f"wl{li}")
                nc.sync.dma_start(out=wl_t[:], in_=wl_ap)
                wr_t = wpool.tile([P, D], dt_mm, tag=f"wr{li}")
                nc.sync.dma_start(out=wr_t[:], in_=wr_ap)
                bl_t = wpool.tile([1, D], dt_mm, tag=f"bl{li}")
                nc.sync.dma_start(out=bl_t[:], in_=bl_ap)
                br_t = wpool.tile([1, D], dt_mm, tag=f"br{li}")
                nc.sync.dma_start(out=br_t[:], in_=br_ap)

                for sl in range(7):
                    st = slabp.tile([P, SLAB], dt_mm, tag="xslab")
                    if li == 0:
                        st8 = slabp.tile([P, SLAB], FP8, tag="xslab8")
                        nc.sync.dma_start(
                            out=st8[:], in_=src_ap[:, sl * SLAB:(sl + 1) * SLAB])
                        nc.vector.tensor_copy(st[:], st8[:])
                    else:
                        nc.sync.dma_start(
                            out=st[:], in_=src_ap[:, sl * SLAB:(sl + 1) * SLAB])
                    for t in range(7):
                        j = sl * 7 + t
                        psl = psA.tile([P, D], F32, tag="psA")
                        nc.tensor.matmul(out=psl[:], lhsT=st[:, t * P:(t + 1) * P],
                                         rhs=wl_t[:], start=True, stop=False)
                        nc.tensor.matmul(out=psl[:], lhsT=ones_t[:], rhs=bl_t[:],
                                         start=False, stop=True)
                        ol = niop.tile([P, D], EDT, tag="xlout")
                        nc.scalar.activation(ol[:], psl[:], AF.Copy)
                        nc.sync.dma_start(out=xl_out[j * P:(j + 1) * P, :], in_=ol[:])
                        psr = psA.tile([P, D], F32, tag="psA")
                        nc.tensor.matmul(out=psr[:], lhsT=st[:, t * P:(t + 1) * P],
                                         rhs=wr_t[:], start=True, stop=False)
                        nc.tensor.matmul(out=psr[:], lhsT=ones_t[:], rhs=br_t[:],
                                         start=False, stop=True)
                        orr = niop.tile([P, D], EDT, tag="xlout")
                        nc.scalar.activation(orr[:], psr[:], AF.Copy)
                        nc.sync.dma_start(out=xr_out[j * P:(j + 1) * P, :], in_=orr[:])

            def edge_phase(li, xl_ap, xr_ap, out_own_ap, out_dt):
                # broadcast att row -> [P, D] via ones-column matmul
                att_row_t = wpool.tile([1, D], EDT, tag=f"attr{li}")
                nc.sync.dma_start(out=att_row_t[:], in_=V(f"attrow{li}"))
                att_ps = psA.tile([P, D], F32, tag="psA")
                nc.tensor.matmul(out=att_ps[:], lhsT=onesrow_e_t[:],
                                 rhs=att_row_t[:], start=True, stop=True)
                att_t = wpool.tile([P, D], EDT, tag=f"att{li}")
                nc.scalar.activation(att_t[:], att_ps[:], AF.Copy)
                bias_t = wpool.tile([P, 1], F32, tag=f"bias{li}")
                nc.sync.dma_start(out=bias_t[:], in_=V(f"biascol{li}"))

                for s in range(NST):
                    tt = int(T[s])
                    # index slices for this super-tile (compact dtypes)
                    is16 = idxp.tile([P, tt], U16, tag="is16")
                    nc.sync.dma_start(
                        out=is16[:], in_=e_sidx[:, off_t[s]:off_t[s] + tt])
                    d8 = idxp.tile([P, tt], I8, tag="d8")
                    nc.sync.dma_start(out=d8[:], in_=e_dloc[:, off_t[s]:off_t[s] + tt])
                    dl_t = idxp.tile([P, tt], EDT, tag="dl")
                    nc.vector.tensor_copy(dl_t[:], d8[:])
                    is_off = idxp.tile([P, tt], I32, tag="is32")
                    nc.vector.tensor_copy(is_off[:], is16[:])
                    # local dst row = relu(dloc + s*128); pads (dloc=-1) land
                    # on a valid in-range row and are masked out by selx
                    dl_f = idxp.tile([P, tt], F32, tag="dlf")
                    nc.vector.tensor_copy(dl_f[:], d8[:])
                    ir_off = idxp.tile([P, tt], I32, tag="ir32")
                    nc.scalar.activation(ir_off[:], dl_f[:], AF.Relu,
                                         bias=stepcol[:, s:s + 1])

                    xlbuf = gathp.tile([P, tt, D], EDT, tag="xlbuf")
                    xrbuf = gathp.tile([P, tt, D], EDT, tag="xrbuf")
                    for t in range(tt):
                        nc.gpsimd.indirect_dma_start(
                            out=xlbuf[:, t, :], out_offset=None, in_=xl_ap[:],
                            in_offset=bass.IndirectOffsetOnAxis(
                                ap=is_off[:, t:t + 1], axis=0))
                        nc.gpsimd.indirect_dma_start(
                            out=xrbuf[:, t, :], out_offset=None, in_=xr_ap[:],
                            in_offset=bass.IndirectOffsetOnAxis(
                                ap=ir_off[:, t:t + 1], axis=0))

                    logits_t = edgep.tile([P, tt], F32, tag="logits")
                    for t in range(tt):
                        xlg_ = xlbuf[:, t, :]
                        xrg_ = xrbuf[:, t, :]
                        t1 = sttp.tile([P, D], EDT, tag="t1")
                        nc.vector.tensor_add(t1[:], xlg_, xrg_)
                        lr = sttp.tile([P, D], EDT, tag="lr")
                        nc.vector.scalar_tensor_tensor(
                            out=lr[:], in0=t1[:], scalar=NEG, in1=t1[:],
                            op0=ALU.mult, op1=ALU.max)
                        junk = sttp.tile([P, D], EDT, tag="junk")
                        nc.vector.scalar_tensor_tensor(
                            out=junk[:], in0=lr[:], scalar=1.0, in1=att_t[:],
                            op0=ALU.mult, op1=ALU.mult,
                            accum_out=logits_t[:, t:t + 1])
                    ex_t = edgep.tile([P, tt], EDT, tag="ex")
                    nc.scalar.activation(ex_t[:], logits_t[:], AF.Exp)

                    psf = psE.tile([P, D], F32, tag="psf")
                    psd = psE.tile([P, 1], F32, tag="psd")
                    for t in range(tt):
                        selx = edgep.tile([P, P], EDT, tag="selx")
                        nc.vector.scalar_tensor_tensor(
                            out=selx[:], in0=iota_t[:], scalar=dl_t[:, t:t + 1],
                            in1=ex_t[:, t:t + 1].to_broadcast([P, P]),
                            op0=ALU.is_equal, op1=ALU.mult)
                        nc.tensor.matmul(out=psf[:], lhsT=selx[:],
                                         rhs=xlbuf[:, t, :],
                                         start=(t == 0), stop=(t == tt - 1))
                        nc.tensor.matmul(out=psd[:], lhsT=selx[:],
                                         rhs=onescol_t[:],
                                         start=(t == 0), stop=False)
                    nc.tensor.matmul(out=psd[:], lhsT=onesrow_e_t[:],
                                     rhs=epsone_t[:], start=False, stop=True)
                    # epilogue
                    rec_t = epip.tile([P, 1], F32, tag="rec")
                    nc.vector.reciprocal(rec_t[:], psd[:])
                    outn = epip.tile([P, D], F32, tag="outn")
                    nc.scalar.activation(outn[:], psf[:], AF.Copy,
                                         scale=rec_t[:])
                    tps = psT.tile([P, D], F32, tag="psT")
                    nc.tensor.transpose(out=tps[:], in_=outn[:], identity=ident_t[:])
                    outT = epip.tile([P, D], out_dt, tag="outT")
                    nc.scalar.activation(outT[:], tps[:], AF.Relu, bias=bias_t[:])
                    nc.sync.dma_start(
                        out=out_own_ap[:, s * P:(s + 1) * P], in_=outT[:])

            # ---------------- layers ----------------
            for li in range(NLAYER):
                if li == 0:
                    node_matmul_phase(V("xTown"), XIN, V("wlt0"), V("wrt0"),
                                      V("blrow0"), V("brrow0"), onesrow_x_t,
                                      xl_own[0], xr[0], 0)
                else:
                    node_matmul_phase(xoTb[li - 1], XDT,
                                      V(f"wltb{li - 1}"), V(f"wrtb{li - 1}"),
                                      V(f"blrowb{li - 1}"), V(f"brrowb{li - 1}"),
                                      onesrow_b_t, xl_own[li], xr[li], li)
                nc.gpsimd.collective_compute(
                    "AllGather", ALU.bypass,
                    replica_groups=[list(range(NCORE))],
                    ins=[xl_own[li][:]], outs=[xlg[li][:]])
                if li < NLAYER - 1:
                    edge_phase(li, xlg[li], xr[li], xoTb[li], XDT)
                else:
                    edge_phase(li, xlg[li], xr[li], xoT2, XIN)

            # ---------------- MLP head (folded: y = x @ Wf.T + bf) ------
            wft_t = wpool.tile([P, DOUT], XIN, tag="wft")
            nc.sync.dma_start(out=wft_t[:], in_=V("wft"))
            bf_t = wpool.tile([1, DOUT], XIN, tag="bfrow")
            nc.sync.dma_start(out=bf_t[:], in_=V("bfrow"))
            for jj in range(NST):
                x3_t = niop.tile([P, P], XIN, tag="x3t")
                nc.sync.dma_start(out=x3_t[:], in_=xoT2[:, jj * P:(jj + 1) * P])
                yps = psA.tile([DOUT, P], F32, tag="psA")
                nc.tensor.matmul(out=yps[:], lhsT=wft_t[:], rhs=x3_t[:],
                                 start=True, stop=False)
                nc.tensor.matmul(out=yps[:], lhsT=bf_t[:], rhs=onesrow_x_t[:],
                                 start=False, stop=True)
                y_t = niop.tile([DOUT, P], YDT, tag="yt")
                nc.scalar.activation(y_t[:], yps[:], AF.Copy)
                nc.sync.dma_start(out=yT[:, jj * P:(jj + 1) * P], in_=y_t[:])

    nc.compile()
    return nc


def _make_in_maps(inputs, ep):
    x = np.asarray(inputs["x"], np.float32)
    Wl = np.asarray(inputs["Wl"], np.float32)
    bl = np.asarray(inputs["bl"], np.float32)
    Wr = np.asarray(inputs["Wr"], np.float32)
    br = np.asarray(inputs["br"], np.float32)
    att = np.asarray(inputs["att"], np.float32)
    bias = np.asarray(inputs["bias"], np.float32)
    W1 = np.asarray(inputs["W1"], np.float32)
    b1 = np.asarray(inputs["b1"], np.float32)
    W2 = np.asarray(inputs["W2"], np.float32)
    b2 = np.asarray(inputs["b2"], np.float32)

    CT = int(ep["T"].sum())
    man, TOTAL = _manifest(CT)

    xTp = np.zeros((P, NP_), ml_dtypes.float8_e4m3fn)
    xTp[:, :N] = x.T.astype(ml_dtypes.float8_e4m3fn)
    common = {
        "wlt0": Wl[0].T.astype(XIN_NP),
        "wrt0": Wr[0].T.astype(XIN_NP),
        "wft": (W2 @ W1).T.astype(XIN_NP),
        "wltb0": Wl[1].T.astype(XDT_NP),
        "wltb1": Wl[2].T.astype(XDT_NP),
        "wrtb0": Wr[1].T.astype(XDT_NP),
        "wrtb1": Wr[2].T.astype(XDT_NP),
        "blrow0": bl[0][None, :].astype(XIN_NP),
        "brrow0": br[0][None, :].astype(XIN_NP),
        "bfrow": (W2 @ b1 + b2)[None, :].astype(XIN_NP),
        "onesrow_x": np.ones((1, P), XIN_NP),
        "blrowb0": bl[1][None, :].astype(XDT_NP),
        "blrowb1": bl[2][None, :].astype(XDT_NP),
        "brrowb0": br[1][None, :].astype(XDT_NP),
        "brrowb1": br[2][None, :].astype(XDT_NP),
        "onesrow_b": np.ones((1, P), XDT_NP),
        "attrow0": att[0][None, :].astype(EDT_NP),
        "attrow1": att[1][None, :].astype(EDT_NP),
        "attrow2": att[2][None, :].astype(EDT_NP),
        "onesrow_e": np.ones((1, P), EDT_NP),
        "onescol": np.ones((P, 1), EDT_NP),
        "epsone": np.full((1, 1), 1e-30, EDT_NP),
        "biascol0": bias[0][:, None].astype(np.float32),
        "biascol1": bias[1][:, None].astype(np.float32),
        "biascol2": bias[2][:, None].astype(np.float32),
    }

    in_maps = []
    for c in range(NCORE):
        vals = dict(common)
        vals["xTown"] = xTp[:, c * PC:(c + 1) * PC]
        vals["sidx"] = ep["srcidx"][c]
        vals["dloc"] = ep["dstloc"][c]
        blob = np.zeros(TOTAL, np.uint8)
        for name, (off, shape, mdt) in man.items():
            a = np.ascontiguousarray(vals[name], dtype=_NPDT[mdt])
            assert a.shape == shape, (name, a.shape, shape)
            raw = np.frombuffer(a.tobytes(), np.uint8)
            blob[off:off + raw.size] = raw
        in_maps.append({"blob": blob.view(np.float32).reshape(1, -1)})
    return in_maps


def _get_compiled(edge_index):
    key = hashlib.md5(np.asarray(edge_index).tobytes()).hexdigest()
    if key not in _CACHE:
        ep = _prep_edges(edge_index)
        nc = _build_program(ep["T"])
        _CACHE[key] = (nc, ep)
    return _CACHE[key]


def _assemble(results):
    y = np.zeros((N, DOUT), np.float32)
    for c in range(NCORE):
        sl = results[c]["yT"].T.astype(np.float32)  # [PC, DOUT]
        lo = c * PC
        hi = min((c + 1) * PC, N)
        if lo < N:
            y[lo:hi] = sl[: hi - lo]
    return y


def kernel(**inputs):
    nc, ep = _get_compiled(inputs["edge_index"])
    in_maps = _make_in_maps(inputs, ep)
    res = run_bass_kernel_spmd(nc, in_maps, core_ids=list(range(NCORE)))
    return _assemble(res.results)
